# revision 1
# baseline (speedup 1.0000x reference)
"""Trainium2 Bass kernel for the attention-scoring MLP (nn_Attn):

    enc = encoder_outputs.transpose(1,0,2)          # [B,S,Hin]
    a1  = tanh(enc @ W1_enc.T + hidden @ W1_hid.T + b1)
    s   = a1 @ W2[0] (+ b2 -- dropped: softmax shift-invariant)
    s   = where(mask, -inf, s)
    out = softmax(s, axis=-1)[:, None, :]           # [B,1,S]

Sharding: data-parallel over batch B=32 across 8 NeuronCores (4 rows
each), weights replicated, no collectives. Per core the main matmul is
computed transposed -- a1T[h, s] = W1_encT.T @ encT per batch row -- so
the (b1 + hidden@W1_hid.T) term rides the ScalarEngine's per-partition
bias port of the tanh activation, and the W2 contraction is a
PSUM-accumulated M=1 matmul over h-tiles. Matmuls run in bf16 (inputs
pre-transposed and converted host-side so all DMAs are contiguous
row-major loads); accumulation is fp32 in PSUM.
"""

import numpy as np
import ml_dtypes

import concourse.bass as bass
import concourse.tile as tile
from concourse import bacc, mybir
from concourse.bass import ds, ts
from concourse.bass_utils import run_bass_kernel_spmd
from concourse.masks import make_identity

N_CORES = 8
B, S, HIN, H = 32, 1024, 1024, 1024
BL = B // N_CORES          # local batch rows per core
P = 128                    # partitions
IT = HIN // P              # contraction tiles
HT = H // P                # output-feature tiles
NT = 512                   # moving-dim tile (s columns per matmul)
SH = S // NT               # s tiles per batch row
F32 = mybir.dt.float32
BF16 = mybir.dt.bfloat16
AF = mybir.ActivationFunctionType
BF = ml_dtypes.bfloat16

_cached_nc = None
LAST_RESULT = None  # BassKernelResults of the most recent run (for test harness)


def _build():
    global _cached_nc
    if _cached_nc is not None:
        return _cached_nc

    nc = bacc.Bacc("TRN2", target_bir_lowering=False, debug=False,
                   num_devices=N_CORES)

    # encT per batch row: [b, i, s]
    enc_ext = nc.dram_tensor("enc", [BL, HIN, S], BF16, kind="ExternalInput").ap()
    # hiddenT: [i, b]
    hidt_ext = nc.dram_tensor("hiddent", [H, BL], BF16, kind="ExternalInput").ap()
    mneg_ext = nc.dram_tensor("maskneg", [BL * S], F32, kind="ExternalInput").ap()
    # W1 split + transposed: [i, h]
    w1e_ext = nc.dram_tensor("w1e", [HIN, H], BF16, kind="ExternalInput").ap()
    w1h_ext = nc.dram_tensor("w1h", [H, H], BF16, kind="ExternalInput").ap()
    b1_ext = nc.dram_tensor("b1", [H], F32, kind="ExternalInput").ap()
    w2_ext = nc.dram_tensor("w2", [H], BF16, kind="ExternalInput").ap()
    out_ext = nc.dram_tensor("out", [BL, S], F32, kind="ExternalOutput").ap()

    with tile.TileContext(nc) as tc:
        with (
            tc.tile_pool(name="consts", bufs=1) as consts,
            tc.tile_pool(name="encp", bufs=3) as encp,
            tc.tile_pool(name="thp", bufs=7) as thp,
            tc.tile_pool(name="pap", bufs=2, space="PSUM") as pap,
            tc.tile_pool(name="pscp", bufs=2, space="PSUM") as pscp,
            tc.tile_pool(name="psA", bufs=1, space="PSUM") as psA,
            tc.tile_pool(name="psT", bufs=2, space="PSUM") as psTp,
        ):
            # ---- PE warmup: ~4us of junk matmuls with no DMA deps so the
            # HAM clock-gate is already at 8/8 when the real matmuls arrive.
            warm_sb = consts.tile([P, NT], BF16)
            nc.gpsimd.memset(warm_sb[:], 0.0)
            warm_ps = pap.tile([P, NT], F32, tag="pa1")
            for _ in range(10):
                nc.tensor.matmul(warm_ps[:], warm_sb[:, 0:P], warm_sb[:],
                                 start=True, stop=True)

            # ---- resident weights/constants ----
            # DMA emission order = ring service order: first-needed first.
            # w1e_sb[p, it*H + h] = W1[h, it*128+p]  == w1e_ext[it*128+p, h]
            # One DMA per h-tile: the ht=0 matmul group only waits for 256KB
            # of weights instead of the whole 2MB.
            w1e_t = []
            for it in range(IT):
                w = consts.tile([P, H], BF16, tag=f"w1e{it}")
                nc.sync.dma_start(w[:], w1e_ext[ds(it * P, P), :])
                w1e_t.append(w)
            hT_sb = consts.tile([P, IT * BL], BF16)
            for it in range(IT):
                nc.sync.dma_start(hT_sb[:, ts(it, BL)], hidt_ext[ds(it * P, P), :])
            # first enc block is prefetched here, before w1h (phase A can
            # wait). Split into per-it tiles so the very first matmul only
            # needs w1e[0]+enc0[0] (~256KB), not the whole 3MB preload.
            enc0_t = []
            for it in range(IT):
                e = encp.tile([P, NT], BF16, tag=f"enc0_{it}")
                nc.scalar.dma_start(e[:], enc_ext[0, ds(it * P, P), ds(0, NT)])
                enc0_t.append(e)
            w1h_t = []
            for it in range(IT):
                w = consts.tile([P, H], BF16, tag=f"w1h{it}")
                nc.scalar.dma_start(w[:], w1h_ext[ds(it * P, P), :])
                w1h_t.append(w)
            b1T_sb = consts.tile([P, HT], F32)
            nc.sync.dma_start(b1T_sb[:], b1_ext.rearrange("(ht p) -> p ht", p=P))
            w2T_sb = consts.tile([P, HT], BF16)
            nc.sync.dma_start(w2T_sb[:], w2_ext.rearrange("(ht p) -> p ht", p=P))
            mneg_sb = consts.tile([1, BL * S], F32)
            nc.sync.dma_start(mneg_sb[:], mneg_ext[:])
            ident_sb = consts.tile([BL, BL], F32)
            make_identity(nc, ident_sb[:])
            # W2 as a padded [128,128] stationary per h-tile (column 0 = w2
            # chunk, rest zero) so the scores matmul keeps the same PE config
            # as the main matmuls; only row 0 of its PSUM output is used.
            w2pad = consts.tile([P, HT * P], BF16)
            nc.gpsimd.memset(w2pad[:], 0.0)
            for ht in range(HT):
                nc.vector.tensor_copy(w2pad[:, ds(ht * P, 1)], w2T_sb[:, ds(ht, 1)])

            bias_sb = consts.tile([P, HT * BL], F32)   # [p, ht*BL+b]
            hterm_sb = consts.tile([BL, H], F32)
            scores_sb = consts.tile([1, BL * S], F32)
            c40 = consts.tile([1, 1], F32)
            nc.gpsimd.memset(c40[:], -40.0)
            exps = consts.tile([1, BL * S], F32)
            ssum = consts.tile([1, BL * SH], F32)
            rcp = consts.tile([1, BL], F32)
            attn = consts.tile([1, BL * S], F32)

            # ---- phase A: h_term[b,h] = hidden @ W1_hid.T; bias = h_termT + b1T
            pht = psA.tile([BL, H], F32)
            for it in range(IT):
                lhs = hT_sb[:, ts(it, BL)]
                nc.tensor.matmul(pht[:, 0:NT], lhs,
                                 w1h_t[it][:, ds(0, NT)],
                                 start=(it == 0), stop=(it == IT - 1))
                nc.tensor.matmul(pht[:, NT:H], lhs,
                                 w1h_t[it][:, ds(NT, NT)],
                                 start=(it == 0), stop=(it == IT - 1))
            nc.scalar.copy(hterm_sb[:], pht[:])
            for ht in range(HT):
                ptT = psTp.tile([P, BL], F32)
                nc.tensor.transpose(ptT[:], hterm_sb[:, ts(ht, P)], ident_sb[:])
                nc.vector.tensor_scalar_add(bias_sb[:, ts(ht, BL)], ptT[:],
                                            b1T_sb[:, ds(ht, 1)])

            # ---- phase B: per (b, s-half) tile of 512 sequence positions
            for t in range(BL * SH):
                b, sh = divmod(t, SH)
                # encT block: enc_sb[p, it*NT + s] = enc_ext[b, it*128+p, sh*NT+s]
                if t == 0:
                    enc_sb = None
                else:
                    enc_sb = encp.tile([P, IT * NT], BF16, tag="enc")
                    # t==1 rides the scalar ring (startup overlap with w1e on
                    # sync); steady-state tiles use the otherwise-idle sync
                    # ring so DMA triggers never serialize against tanh on ACT.
                    eng = nc.scalar if t == 1 else nc.sync
                    for it in range(IT):
                        eng.dma_start(
                            enc_sb[:, ts(it, NT)],
                            enc_ext[b, ds(it * P, P), ds(sh * NT, NT)],
                        )
                psc = pscp.tile([P, NT], F32)
                # Delay the scores matmuls so a late bias (phase A is still
                # streaming during t=0) never stalls the in-order PE.
                delay = 4 if t == 0 else (1 if t == BL * SH - 1 else 3)
                pending = []
                for ht in range(HT):
                    pa1 = pap.tile([P, NT], F32, tag="pa1")
                    for it in range(IT):
                        rhs = enc0_t[it][:] if t == 0 else enc_sb[:, ts(it, NT)]
                        nc.tensor.matmul(
                            pa1[:],
                            w1e_t[it][:, ds(ht * P, P)],
                            rhs,
                            start=(it == 0), stop=(it == IT - 1),
                        )
                    th = thp.tile([P, NT], BF16)
                    nc.scalar.activation(th[:], pa1[:], AF.Tanh,
                                         bias=bias_sb[:, ds(ht * BL + b, 1)],
                                         scale=1.0)
                    pending.append((th, ht))
                    if len(pending) > delay:
                        pth, pht_idx = pending.pop(0)
                        nc.tensor.matmul(psc[:], w2pad[:, ds(pht_idx * P, P)],
                                         pth[:],
                                         start=(pht_idx == 0),
                                         stop=(pht_idx == HT - 1))
                for pth, pht_idx in pending:
                    nc.tensor.matmul(psc[:], w2pad[:, ds(pht_idx * P, P)],
                                     pth[:], start=(pht_idx == 0),
                                     stop=(pht_idx == HT - 1))
                # scores += mask * -1e30   (scores_sb[0, t*NT:] == scores[b, sh*NT:])
                nc.vector.tensor_add(scores_sb[0:1, ds(t * NT, NT)], psc[0:1, :],
                                     mneg_sb[0:1, ds(t * NT, NT)])

                # ---- softmax, pipelined per s-half tile.
                # |scores| <= ||W2||_1 <= 32, so exp(s - 40) never overflows
                # and softmax is shift-invariant -- no max-reduce needed.
                nc.scalar.activation(exps[0:1, ds(t * NT, NT)],
                                     scores_sb[0:1, ds(t * NT, NT)],
                                     AF.Exp, bias=c40[0:1, 0:1], scale=1.0,
                                     accum_out=ssum[0:1, ds(t, 1)])
                if sh == SH - 1:
                    # total = sum of the SH per-tile partial sums for row b
                    nc.vector.reduce_sum(rcp[0:1, ds(b, 1)],
                                         ssum[0:1, ds(b * SH, SH)],
                                         axis=mybir.AxisListType.X)
                    nc.vector.reciprocal(rcp[0:1, ds(b, 1)], rcp[0:1, ds(b, 1)])
                    nc.vector.tensor_scalar_mul(attn[0:1, ds(b * S, S)],
                                                exps[0:1, ds(b * S, S)],
                                                rcp[0:1, ds(b, 1)])
                    nc.sync.dma_start(out_ext[b, :], attn[0:1, ds(b * S, S)])

    nc.compile()
    _cached_nc = nc
    return nc


def kernel(hidden, encoder_outputs, mask, W1, b1, W2, b2):
    global LAST_RESULT
    nc = _build()

    enc = np.asarray(encoder_outputs, dtype=np.float32)
    # [S,B,Hin] -> [B,Hin,S] in bf16 so per-core DMAs are contiguous
    enc_t = np.ascontiguousarray(np.transpose(enc, (1, 2, 0)).astype(BF))
    hid_t = np.ascontiguousarray(np.asarray(hidden, dtype=np.float32).T.astype(BF))  # [H,B]
    maskneg = np.where(np.asarray(mask, dtype=bool), np.float32(-1e30),
                       np.float32(0.0)).astype(np.float32)
    W1 = np.asarray(W1, dtype=np.float32)
    w1e = np.ascontiguousarray(W1[:, :HIN].T.astype(BF))   # [Hin, H]
    w1h = np.ascontiguousarray(W1[:, HIN:].T.astype(BF))   # [H, H]
    b1 = np.ascontiguousarray(np.asarray(b1, dtype=np.float32).reshape(H))
    w2 = np.ascontiguousarray(np.asarray(W2, dtype=np.float32).reshape(H).astype(BF))

    in_maps = []
    for c in range(N_CORES):
        sl = slice(c * BL, (c + 1) * BL)
        in_maps.append({
            "enc": np.ascontiguousarray(enc_t[sl]),
            "hiddent": np.ascontiguousarray(hid_t[:, sl]),
            "maskneg": np.ascontiguousarray(maskneg[sl].reshape(-1)),
            "w1e": w1e,
            "w1h": w1h,
            "b1": b1,
            "w2": w2,
        })

    res = run_bass_kernel_spmd(nc, in_maps, core_ids=list(range(N_CORES)))
    LAST_RESULT = res
    out = np.concatenate([res.results[c]["out"] for c in range(N_CORES)], axis=0)
    return np.ascontiguousarray(out[:, None, :].astype(np.float32))



# revision 3
# speedup vs baseline: 1.8978x; 1.8978x over previous
"""Trainium2 Bass kernel for the attention-scoring MLP (nn_Attn):

    enc = encoder_outputs.transpose(1,0,2)          # [B,S,Hin]
    a1  = tanh(enc @ W1_enc.T + hidden @ W1_hid.T + b1)
    s   = a1 @ W2[0] (+ b2 -- dropped: softmax shift-invariant)
    s   = where(mask, -inf, s)
    out = softmax(s, axis=-1)[:, None, :]           # [B,1,S]

Sharding: data-parallel over batch B=32 across 8 NeuronCores (4 rows
each), weights replicated, no collectives.

Two levers over the naive dense form:
  1. Mask compaction. ~half the sequence positions are masked to -inf
     before the softmax, so their scores are never needed. The host
     gathers, per batch row, only the unmasked enc columns (padded to a
     uniform per-row capacity, multiple of 64) and the kernel computes
     scores only for those. Padded slots carry an additive -1e30 so
     their exp is exactly 0; the host scatters the compact attn back to
     full [B,1,S] with zeros at masked positions.
  2. fp8 (e4m3) DoubleRow matmuls for the dominant enc @ W1_enc.T
     contraction: both operands are absmax-scaled to fp8 on the host,
     pairs of 128-deep k-subtiles are fed per matmul (2 rows/cycle),
     and the dequant scale rides the ScalarEngine tanh activation's
     scale port. Accumulation stays fp32 in PSUM; the (b1 +
     hidden@W1_hid.T) term rides the tanh's per-partition bias port,
     computed on device in bf16 (phase A). The scores contraction
     (a1 @ W2) stays bf16.
"""

import numpy as np
import ml_dtypes

import concourse.bass as bass
import concourse.tile as tile
from concourse import bacc, mybir
from concourse.bass import ds, ts
from concourse.bass_utils import run_bass_kernel_spmd
from concourse.masks import make_identity

N_CORES = 8
B, S, HIN, H = 32, 1024, 1024, 1024
BL = B // N_CORES          # local batch rows per core
P = 128                    # partitions
IT = HIN // P              # contraction subtiles (128 deep each)
HT = H // P                # output-feature tiles
F32 = mybir.dt.float32
BF16 = mybir.dt.bfloat16
FP8 = mybir.dt.float8e4
AF = mybir.ActivationFunctionType
DR = mybir.MatmulPerfMode.DoubleRow
BF = ml_dtypes.bfloat16
F8 = ml_dtypes.float8_e4m3   # TRN fp8e4: IEEE e4m3, max finite +-240

_cached = {}
LAST_RESULT = None  # BassKernelResults of the most recent run (for test harness)


def _structure(padb):
    """Tile structure for per-row compact width `padb` (multiple of 64).

    Returns (tiles, chunks, total):
      tiles:  ordered list; each tile is (width, [(b, start_in_b, seg_w,
              dev_off, slot)]) -- shared-tail tiles first, then per-b
              full 512 chunks in b order.
      chunks: per b, list of (dev_off, width, start_in_b, slot) in
              ascending start order.
      total:  device column count (sum of tile widths).
    """
    per_b = [512] * (padb // 512)
    if padb % 512:
        per_b.append(padb % 512)
    if not per_b:
        per_b = [padb]
    kmax = len(per_b)

    tail_w = per_b[-1] if (kmax > 1 and per_b[-1] < 512) else None
    tiles = []
    chunks = [[] for _ in range(BL)]
    dev = 0
    if tail_w is not None:
        # pack the 4 per-b tails into shared tiles of total width <= 512
        group = []
        gw = 0
        start_in_b = 512 * (kmax - 1)
        for b in range(BL):
            if gw + tail_w > 512:
                tiles.append((gw, group))
                group, gw = [], 0
            group.append((b, start_in_b, tail_w, dev, b * kmax + kmax - 1))
            chunks[b].append((dev, tail_w, start_in_b, b * kmax + kmax - 1))
            dev += tail_w
            gw += tail_w
        tiles.append((gw, group))
    n_main = kmax - 1 if tail_w is not None else kmax
    for b in range(BL):
        for k in range(n_main):
            w = per_b[k]
            tiles.append((w, [(b, 512 * k, w, dev, b * kmax + k)]))
            chunks[b].append((dev, w, 512 * k, b * kmax + k))
            dev += w
    for b in range(BL):
        chunks[b].sort(key=lambda c: c[2])
    return tiles, chunks, dev, kmax


def _build(padb, scl):
    key = (padb, scl)
    if key in _cached:
        return _cached[key]

    tiles, chunks, TOTAL, kmax = _structure(padb)
    # last tile index that touches each b (for softmax finalization)
    last_tile_of_b = {}
    for t, (_, segs) in enumerate(tiles):
        for b, *_ in segs:
            last_tile_of_b[b] = t
    NTILES = len(tiles)

    nc = bacc.Bacc("TRN2", target_bir_lowering=False, debug=False,
                   num_devices=N_CORES)

    # compact encT, device-column layout: [i, dev_col]
    enc_ext = nc.dram_tensor("enc", [HIN, TOTAL], FP8, kind="ExternalInput").ap()
    # hiddenT: [i, b]
    hidt_ext = nc.dram_tensor("hiddent", [H, BL], BF16, kind="ExternalInput").ap()
    mneg_ext = nc.dram_tensor("maskneg", [TOTAL], F32, kind="ExternalInput").ap()
    # W1 split + transposed: [i, h]
    w1e_ext = nc.dram_tensor("w1e", [HIN, H], FP8, kind="ExternalInput").ap()
    w1h_ext = nc.dram_tensor("w1h", [H, H], BF16, kind="ExternalInput").ap()
    b1_ext = nc.dram_tensor("b1", [H], F32, kind="ExternalInput").ap()
    w2_ext = nc.dram_tensor("w2", [H], BF16, kind="ExternalInput").ap()
    out_ext = nc.dram_tensor("out", [TOTAL], F32, kind="ExternalOutput").ap()

    with tile.TileContext(nc) as tc:
        with (
            tc.tile_pool(name="consts", bufs=1) as consts,
            tc.tile_pool(name="encp", bufs=3) as encp,
            tc.tile_pool(name="thp", bufs=7) as thp,
            tc.tile_pool(name="pap", bufs=2, space="PSUM") as pap,
            tc.tile_pool(name="pscp", bufs=2, space="PSUM") as pscp,
            tc.tile_pool(name="psA", bufs=1, space="PSUM") as psA,
            tc.tile_pool(name="psT", bufs=2, space="PSUM") as psTp,
        ):
            # ---- PE warmup: junk matmuls with no DMA deps so the HAM
            # clock-gate is already at 8/8 when the real matmuls arrive.
            warm_sb = consts.tile([P, 512], BF16)
            nc.gpsimd.memset(warm_sb[:], 0.0)
            warm_ps = pap.tile([P, 512], F32, tag="pa1")
            for _ in range(10):
                nc.tensor.matmul(warm_ps[:], warm_sb[:, 0:P], warm_sb[:],
                                 start=True, stop=True)

            # ---- resident weights/constants ----
            # DMA emission order = ring service order: first-needed first.
            # One DMA per k-subtile so the first matmul group only waits
            # for 128KB of weights.
            w1e_sb = consts.tile([P, IT, H], FP8)
            for it in range(IT):
                nc.sync.dma_start(w1e_sb[:, it, :], w1e_ext[ds(it * P, P), :])
            hT_sb = consts.tile([P, IT * BL], BF16)
            for it in range(IT):
                nc.sync.dma_start(hT_sb[:, ts(it, BL)], hidt_ext[ds(it * P, P), :])
            # first tile's enc rides the scalar ring (startup overlap with
            # w1e on sync), before w1h which is only needed for phase A.
            w0 = tiles[0][0]
            enc0_sb = encp.tile([P, IT, w0], FP8, tag="enc0")
            for it in range(IT):
                nc.scalar.dma_start(
                    enc0_sb[:, it, :],
                    enc_ext[ds(it * P, P), ds(tiles[0][1][0][3], w0)])
            w1h_t = []
            for it in range(IT):
                w = consts.tile([P, H], BF16, tag=f"w1h{it}")
                nc.scalar.dma_start(w[:], w1h_ext[ds(it * P, P), :])
                w1h_t.append(w)
            b1T_sb = consts.tile([P, HT], F32)
            nc.sync.dma_start(b1T_sb[:], b1_ext.rearrange("(ht p) -> p ht", p=P))
            w2T_sb = consts.tile([P, HT], BF16)
            nc.sync.dma_start(w2T_sb[:], w2_ext.rearrange("(ht p) -> p ht", p=P))
            mneg_sb = consts.tile([1, TOTAL], F32)
            nc.sync.dma_start(mneg_sb[:], mneg_ext[:])
            ident_sb = consts.tile([BL, BL], F32)
            make_identity(nc, ident_sb[:])
            # W2 as a padded [128,128] stationary per h-tile (column 0 = w2
            # chunk, rest zero) so the scores matmul keeps the same PE config
            # as the main matmuls; only row 0 of its PSUM output is used.
            w2pad = consts.tile([P, HT * P], BF16)
            nc.gpsimd.memset(w2pad[:], 0.0)
            for ht in range(HT):
                nc.vector.tensor_copy(w2pad[:, ds(ht * P, 1)], w2T_sb[:, ds(ht, 1)])

            bias_sb = consts.tile([P, HT * BL], F32)   # [p, ht*BL+b]
            hterm_sb = consts.tile([BL, H], F32)
            scores_sb = consts.tile([1, TOTAL], F32)
            c40 = consts.tile([1, 1], F32)
            nc.gpsimd.memset(c40[:], -40.0)
            exps = consts.tile([1, TOTAL], F32)
            ssum = consts.tile([1, BL * kmax], F32)
            rcp = consts.tile([1, BL], F32)
            attn = consts.tile([1, TOTAL], F32)

            # ---- phase A: h_term[b,h] = hidden @ W1_hid.T; bias = h_termT + b1T
            pht = psA.tile([BL, H], F32)
            for it in range(IT):
                lhs = hT_sb[:, ts(it, BL)]
                nc.tensor.matmul(pht[:, 0:512], lhs,
                                 w1h_t[it][:, ds(0, 512)],
                                 start=(it == 0), stop=(it == IT - 1))
                nc.tensor.matmul(pht[:, 512:H], lhs,
                                 w1h_t[it][:, ds(512, 512)],
                                 start=(it == 0), stop=(it == IT - 1))
            nc.scalar.copy(hterm_sb[:], pht[:])
            for ht in range(HT):
                ptT = psTp.tile([P, BL], F32)
                nc.tensor.transpose(ptT[:], hterm_sb[:, ts(ht, P)], ident_sb[:])
                nc.vector.tensor_scalar_add(bias_sb[:, ts(ht, BL)], ptT[:],
                                            b1T_sb[:, ds(ht, 1)])

            # ---- main loop over compact column tiles
            for t, (W, segs) in enumerate(tiles):
                toff = segs[0][3]
                if t == 0:
                    enc_sb = enc0_sb
                else:
                    enc_sb = encp.tile([P, IT, W], FP8, tag="enc")
                    # t==1 rides the scalar ring (startup overlap with w1e
                    # on sync); steady-state tiles use the otherwise-idle
                    # sync ring so DMA triggers never serialize against
                    # tanh on ACT.
                    eng = nc.scalar if t == 1 else nc.sync
                    for it in range(IT):
                        eng.dma_start(enc_sb[:, it, :],
                                      enc_ext[ds(it * P, P), ds(toff, W)])
                psc = pscp.tile([P, W], F32, tag="psc")
                # Delay the scores matmuls so a late bias (phase A is still
                # streaming during t=0) never stalls the in-order PE.
                delay = 4 if t == 0 else (1 if t == NTILES - 1 else 3)
                pending = []
                for ht in range(HT):
                    pa1 = pap.tile([P, W], F32, tag="pa1")
                    for kp in range(IT // 2):
                        nc.tensor.matmul(
                            pa1[:],
                            w1e_sb[:, ds(2 * kp, 2), ds(ht * P, P)],
                            enc_sb[:, ds(2 * kp, 2), :],
                            start=(kp == 0), stop=(kp == IT // 2 - 1),
                            perf_mode=DR,
                        )
                    th = thp.tile([P, W], BF16)
                    for (b, _, sw, _, _), so in zip(segs, _seg_offs(segs)):
                        nc.scalar.activation(th[:, ds(so, sw)],
                                             pa1[:, ds(so, sw)], AF.Tanh,
                                             bias=bias_sb[:, ds(ht * BL + b, 1)],
                                             scale=scl)
                    pending.append((th, ht))
                    if len(pending) > delay:
                        pth, pht_idx = pending.pop(0)
                        nc.tensor.matmul(psc[:], w2pad[:, ds(pht_idx * P, P)],
                                         pth[:],
                                         start=(pht_idx == 0),
                                         stop=(pht_idx == HT - 1))
                for pth, pht_idx in pending:
                    nc.tensor.matmul(psc[:], w2pad[:, ds(pht_idx * P, P)],
                                     pth[:], start=(pht_idx == 0),
                                     stop=(pht_idx == HT - 1))
                # scores += mask * -1e30 (padding slots killed here too)
                nc.vector.tensor_add(scores_sb[0:1, ds(toff, W)], psc[0:1, :],
                                     mneg_sb[0:1, ds(toff, W)])

                # ---- softmax, pipelined per tile.
                # |scores| <= ||W2||_1 <= 32, so exp(s - 40) never overflows
                # and softmax is shift-invariant -- no max-reduce needed.
                for (b, _, sw, doff, slot), so in zip(segs, _seg_offs(segs)):
                    nc.scalar.activation(exps[0:1, ds(doff, sw)],
                                         scores_sb[0:1, ds(doff, sw)],
                                         AF.Exp, bias=c40[0:1, 0:1], scale=1.0,
                                         accum_out=ssum[0:1, ds(slot, 1)])
                for b in range(BL):
                    if last_tile_of_b[b] != t:
                        continue
                    nc.vector.reduce_sum(rcp[0:1, ds(b, 1)],
                                         ssum[0:1, ds(b * kmax, kmax)],
                                         axis=mybir.AxisListType.X)
                    nc.vector.reciprocal(rcp[0:1, ds(b, 1)], rcp[0:1, ds(b, 1)])
                    for doff, cw, _, _ in chunks[b]:
                        nc.vector.tensor_scalar_mul(attn[0:1, ds(doff, cw)],
                                                    exps[0:1, ds(doff, cw)],
                                                    rcp[0:1, ds(b, 1)])
                        nc.sync.dma_start(out_ext[ds(doff, cw)],
                                          attn[0:1, ds(doff, cw)])

    nc.compile()
    _cached[key] = (nc, tiles, chunks, TOTAL, kmax)
    return _cached[key]


def _seg_offs(segs):
    offs = []
    o = 0
    for _, _, w, _, _ in segs:
        offs.append(o)
        o += w
    return offs


def kernel(hidden, encoder_outputs, mask, W1, b1, W2, b2):
    global LAST_RESULT

    mask = np.asarray(mask, dtype=bool)
    counts = (~mask).sum(axis=1)
    padb = min(-(-int(counts.max()) // 64) * 64, S)

    enc = np.asarray(encoder_outputs, dtype=np.float32)
    enc_t = np.transpose(enc, (1, 2, 0))               # [B, Hin, S] fp32 view
    s_e = 240.0 / max(float(np.abs(enc).max()), 1e-30)
    W1 = np.asarray(W1, dtype=np.float32)
    w1e_f = W1[:, :HIN].T                              # [Hin, H]
    s_w = 240.0 / max(float(np.abs(w1e_f).max()), 1e-30)
    nc, tiles, chunks, TOTAL, kmax = _build(padb, float(1.0 / (s_e * s_w)))
    w1e = np.ascontiguousarray(np.clip(w1e_f * s_w, -240, 240)).astype(F8)
    w1h = np.ascontiguousarray(W1[:, HIN:].T.astype(BF))   # [H, H]
    hid_t = np.ascontiguousarray(np.asarray(hidden, dtype=np.float32).T.astype(BF))
    b1 = np.ascontiguousarray(np.asarray(b1, dtype=np.float32).reshape(H))
    w2 = np.ascontiguousarray(np.asarray(W2, dtype=np.float32).reshape(H).astype(BF))

    in_maps = []
    dev_idx = {}
    for c in range(N_CORES):
        enc_dev = np.zeros((HIN, TOTAL), dtype=F8)
        mneg = np.full(TOTAL, -1e30, dtype=np.float32)
        for b in range(BL):
            gb = c * BL + b
            idx = np.nonzero(~mask[gb])[0]
            cb = len(idx)
            encg = np.clip(enc_t[gb][:, idx] * s_e, -240, 240).astype(F8)
            cols = np.empty(cb, dtype=np.int64)
            for doff, cw, start, _ in chunks[b]:
                real = min(max(cb - start, 0), cw)
                if real > 0:
                    enc_dev[:, doff:doff + real] = encg[:, start:start + real]
                    mneg[doff:doff + real] = 0.0
                    cols[start:start + real] = np.arange(doff, doff + real)
            dev_idx[gb] = (idx, cols)
        sl = slice(c * BL, (c + 1) * BL)
        in_maps.append({
            "enc": enc_dev,
            "hiddent": np.ascontiguousarray(hid_t[:, sl]),
            "maskneg": mneg,
            "w1e": w1e,
            "w1h": w1h,
            "b1": b1,
            "w2": w2,
        })

    res = run_bass_kernel_spmd(nc, in_maps, core_ids=list(range(N_CORES)))
    LAST_RESULT = res
    out = np.zeros((B, S), dtype=np.float32)
    for c in range(N_CORES):
        attn_dev = res.results[c]["out"]
        for b in range(BL):
            gb = c * BL + b
            idx, cols = dev_idx[gb]
            out[gb, idx] = attn_dev[cols]
    return np.ascontiguousarray(out[:, None, :])


# revision 7
# speedup vs baseline: 2.2241x; 1.1719x over previous
"""Trainium2 Bass kernel for the attention-scoring MLP (nn_Attn):

    enc = encoder_outputs.transpose(1,0,2)          # [B,S,Hin]
    a1  = tanh(enc @ W1_enc.T + hidden @ W1_hid.T + b1)
    s   = a1 @ W2[0] (+ b2 -- dropped: softmax shift-invariant)
    s   = where(mask, -inf, s)
    out = softmax(s, axis=-1)[:, None, :]           # [B,1,S]

Sharding: data-parallel over batch B=32 across 8 NeuronCores (4 rows
each), weights replicated, no collectives.

Levers over the naive dense form:
  1. Mask compaction. ~half the sequence positions are masked to -inf
     before the softmax, so their scores are never needed. The host
     gathers, per batch row, only the unmasked enc columns (padded to a
     uniform per-row capacity, multiple of 64) and the kernel computes
     scores only for those. Padded slots carry an additive -1e30 so
     their exp is exactly 0; the host scatters the compact attn back to
     full [B,1,S] with zeros at masked positions. Sub-512 per-row tails
     are packed together into shared tiles.
  2. fp8 (e4m3) DoubleRow matmuls everywhere on the PE: operands are
     absmax-scaled to fp8 on the host, pairs of 128-deep k-subtiles are
     fed per matmul (2 rows/cycle), and dequant scales ride the
     ScalarEngine activation scale ports. Accumulation stays fp32 in
     PSUM.
  3. One DMA per tensor/tile (3D access patterns, host pre-laid as
     [128, k-subtile, col]): DMA trigger instructions cost ~600ns on
     the issuing engine, so per-subtile DMAs serialized ~25us of
     startup.
"""

import numpy as np
import ml_dtypes

import concourse.bass as bass
import concourse.tile as tile
from concourse import bacc, mybir
from concourse.bass import ds, ts
from concourse.bass_utils import run_bass_kernel_spmd
from concourse.masks import make_identity

N_CORES = 8
B, S, HIN, H = 32, 1024, 1024, 1024
BL = B // N_CORES          # local batch rows per core
P = 128                    # partitions
IT = HIN // P              # contraction subtiles (128 deep each)
HT = H // P                # output-feature tiles
F32 = mybir.dt.float32
BF16 = mybir.dt.bfloat16
FP8 = mybir.dt.float8e4
AF = mybir.ActivationFunctionType
DR = mybir.MatmulPerfMode.DoubleRow
BF = ml_dtypes.bfloat16
F8 = ml_dtypes.float8_e4m3   # TRN fp8e4: IEEE e4m3, max finite +-240

_cached = {}
LAST_RESULT = None  # BassKernelResults of the most recent run (for test harness)


def _structure(padb):
    """Tile structure for per-row compact width `padb` (multiple of 64).

    Returns (tiles, chunks, total, kmax):
      tiles:  ordered list; each tile is (width, [(b, start_in_b, seg_w,
              dev_off, slot)]).  Order: b0's full-width chunks, then the
              shared tail tiles (packed sub-512 per-b tails), then
              b1..b3's full chunks -- so phase A overlaps a big tile and
              each b's softmax finalization lands right after its last
              chunk.
      chunks: per b, list of (dev_off, width, start_in_b, slot) in
              ascending start order.
      total:  device column count (sum of tile widths).
    """
    per_b = [512] * (padb // 512)
    if padb % 512:
        per_b.append(padb % 512)
    kmax = len(per_b)

    tail_w = per_b[-1] if (kmax > 1 and per_b[-1] < 512) else None
    n_main = kmax - 1 if tail_w is not None else kmax

    tiles = []
    chunks = [[] for _ in range(BL)]
    dev = 0

    def add_main(b):
        nonlocal dev
        for k in range(n_main):
            w = per_b[k]
            tiles.append((w, [(b, 512 * k, w, dev, b * kmax + k)]))
            chunks[b].append((dev, w, 512 * k, b * kmax + k))
            dev += w

    add_main(0)
    if tail_w is not None:
        group, gw = [], 0
        start_in_b = 512 * (kmax - 1)
        for b in range(BL):
            if gw + tail_w > 512:
                tiles.append((gw, group))
                group, gw = [], 0
            group.append((b, start_in_b, tail_w, dev, b * kmax + kmax - 1))
            chunks[b].append((dev, tail_w, start_in_b, b * kmax + kmax - 1))
            dev += tail_w
            gw += tail_w
        tiles.append((gw, group))
    for b in range(1, BL):
        add_main(b)
    for b in range(BL):
        chunks[b].sort(key=lambda c: c[2])
    return tiles, chunks, dev, kmax


def _build(padb, scl_main, scl_hid, scl_exp):
    key = (padb, scl_main, scl_hid, scl_exp)
    if key in _cached:
        return _cached[key]

    tiles, chunks, TOTAL, kmax = _structure(padb)
    last_tile_of_b = {}
    for t, (_, segs) in enumerate(tiles):
        for b, *_ in segs:
            last_tile_of_b[b] = t
    NTILES = len(tiles)
    # widths of the shared-tail tiles (for the bias_tail buffer)
    tailw = [w for w, segs in tiles if len(segs) > 1]
    TW = max(tailw) if tailw else 0

    nc = bacc.Bacc("TRN2", target_bir_lowering=False, debug=False,
                   num_devices=N_CORES)

    # compact encT, device-column layout, pre-arranged [p, it, col]
    enc_ext = nc.dram_tensor("enc", [P, IT, TOTAL], FP8, kind="ExternalInput").ap()
    # hiddenT, [p, it, b] padded to 16 along b
    hidt_ext = nc.dram_tensor("hiddent", [P, IT, 16], FP8, kind="ExternalInput").ap()
    mneg_ext = nc.dram_tensor("maskneg", [TOTAL], F32, kind="ExternalInput").ap()
    # W1 split, transposed, fp8-scaled, pre-arranged [p, it, h]
    w1e_ext = nc.dram_tensor("w1e", [P, IT, H], FP8, kind="ExternalInput").ap()
    w1h_ext = nc.dram_tensor("w1h", [P, IT, H], FP8, kind="ExternalInput").ap()
    b1_ext = nc.dram_tensor("b1", [H], F32, kind="ExternalInput").ap()
    # W2 padded stationary [p, ht, 128] (col 0 = scaled w2 chunk, rest 0)
    w2p_ext = nc.dram_tensor("w2p", [P, HT, P], FP8, kind="ExternalInput").ap()
    out_ext = nc.dram_tensor("out", [TOTAL], F32, kind="ExternalOutput").ap()

    with tile.TileContext(nc) as tc:
        with (
            tc.tile_pool(name="consts", bufs=1) as consts,
            tc.tile_pool(name="encp", bufs=3) as encp,
            tc.tile_pool(name="thp", bufs=4) as thp,
            tc.tile_pool(name="pap", bufs=2, space="PSUM") as pap,
            tc.tile_pool(name="pscp", bufs=2, space="PSUM") as pscp,
            tc.tile_pool(name="psA", bufs=1, space="PSUM") as psA,
            tc.tile_pool(name="psT", bufs=2, space="PSUM") as psTp,
        ):
            # ---- PE warmup: junk matmuls with no DMA deps so the HAM
            # clock-gate is already at 8/8 when the real matmuls arrive.
            warm_sb = consts.tile([P, 512], BF16)
            nc.gpsimd.memset(warm_sb[:], 0.0)
            warm_ps = pap.tile([P, 512], F32, tag="pa1")
            for _ in range(10):
                nc.tensor.matmul(warm_ps[:], warm_sb[:, 0:P], warm_sb[:],
                                 start=True, stop=True)

            # ---- resident weights/constants (one DMA each) ----
            # Emission order = ring service order: first-needed first.
            w1e_sb = consts.tile([P, IT, H], FP8)
            nc.sync.dma_start(w1e_sb[:], w1e_ext[:])
            # first tile's enc rides the scalar ring (startup overlap
            # with w1e on sync), before w1h (phase A can wait a bit).
            w0 = tiles[0][0]
            enc0_sb = consts.tile([P, IT, w0], FP8, tag="enc0")
            nc.scalar.dma_start(enc0_sb[:],
                                enc_ext[:, :, ds(tiles[0][1][0][3], w0)])
            w1h_sb = consts.tile([P, IT, H], FP8)
            nc.scalar.dma_start(w1h_sb[:], w1h_ext[:])
            hT_sb = consts.tile([P, IT, 16], FP8)
            nc.sync.dma_start(hT_sb[:], hidt_ext[:])
            b1T_sb = consts.tile([P, HT], F32)
            nc.sync.dma_start(b1T_sb[:], b1_ext.rearrange("(ht p) -> p ht", p=P))
            w2pad = consts.tile([P, HT, P], FP8)
            nc.sync.dma_start(w2pad[:], w2p_ext[:])
            mneg_sb = consts.tile([1, TOTAL], F32)
            nc.sync.dma_start(mneg_sb[:], mneg_ext[:])
            ident_sb = consts.tile([BL, BL], F32)
            make_identity(nc, ident_sb[:])

            bias_sb = consts.tile([P, HT * BL], F32)   # [p, ht*BL+b]
            bias_tail = consts.tile([P, HT * TW], F32, name="bias_tail") if TW else None
            hterm_sb = consts.tile([BL, H], F32)
            scores_sb = consts.tile([1, TOTAL], F32)
            c40 = consts.tile([1, 1], F32)
            nc.gpsimd.memset(c40[:], -40.0)
            exps = consts.tile([1, TOTAL], F32)
            ssum = consts.tile([1, BL * kmax], F32)
            rcp = consts.tile([1, BL], F32)
            attn = consts.tile([1, TOTAL], F32)
            ttmp = consts.tile([P, TW], F32, name="ttmp") if TW else None

            # ---- phase A: h_term[b,h] = hidden @ W1_hid.T (fp8);
            # bias = h_termT + b1T, dequant via the Copy scale port.
            pht = psA.tile([BL, H], F32)
            for it in range(IT):
                lhs = hT_sb[:, it, 0:BL]
                nc.tensor.matmul(pht[:, 0:512], lhs, w1h_sb[:, it, 0:512],
                                 start=(it == 0), stop=(it == IT - 1))
                nc.tensor.matmul(pht[:, 512:H], lhs, w1h_sb[:, it, 512:H],
                                 start=(it == 0), stop=(it == IT - 1))
            nc.scalar.activation(hterm_sb[:], pht[:], AF.Copy, scale=scl_hid)
            for ht in range(HT):
                ptT = psTp.tile([P, BL], F32)
                nc.tensor.transpose(ptT[:], hterm_sb[:, ts(ht, P)], ident_sb[:])
                nc.vector.tensor_scalar_add(bias_sb[:, ts(ht, BL)], ptT[:],
                                            b1T_sb[:, ds(ht, 1)])
            # per-column bias for the shared-tail tiles (vector-added
            # there, since segments of different b share one tile)
            if TW:
                tw0, segs0 = next((w, s) for w, s in tiles if len(s) > 1)
                offs0 = _seg_offs(segs0)
                # pre-scaled by 1/scl_main: the shared-tile tanh computes
                # tanh((pa1 + bias_tail) * scl_main)
                for ht in range(HT):
                    for (b, _, sw, _, _), so in zip(segs0, offs0):
                        nc.vector.tensor_scalar(
                            bias_tail[:, ds(ht * TW + so, sw)],
                            warm_sb[:, ds(0, sw)],
                            bias_sb[:, ds(ht * BL + b, 1)],
                            1.0 / scl_main,
                            mybir.AluOpType.add,
                            mybir.AluOpType.mult)

            # ---- main loop over compact column tiles
            for t, (W, segs) in enumerate(tiles):
                toff = segs[0][3]
                shared = len(segs) > 1
                if t == 0:
                    enc_sb = enc0_sb
                else:
                    enc_sb = encp.tile([P, IT, W], FP8, tag="enc")
                    eng = nc.scalar if t == 1 else nc.sync
                    eng.dma_start(enc_sb[:], enc_ext[:, :, ds(toff, W)])
                psc = pscp.tile([P, W], F32, tag="psc")
                # Delay the scores matmuls (issued per ht-pair) so a late
                # bias / tanh never stalls the in-order PE.
                delay = 2 if t == 0 else 1
                pending = []
                th = None
                for ht in range(HT):
                    pa1 = pap.tile([P, W], F32, tag="pa1")
                    for kp in range(IT // 2):
                        nc.tensor.matmul(
                            pa1[:],
                            w1e_sb[:, ds(2 * kp, 2), ds(ht * P, P)],
                            enc_sb[:, ds(2 * kp, 2), :],
                            start=(kp == 0), stop=(kp == IT // 2 - 1),
                            perf_mode=DR,
                        )
                    if ht % 2 == 0:
                        th = thp.tile([P, 2, W], FP8)
                    if shared:
                        nc.vector.tensor_add(ttmp[:, 0:W], pa1[:],
                                             bias_tail[:, ds(ht * TW, W)])
                        nc.scalar.activation(th[:, ds(ht % 2, 1), :],
                                             ttmp[:, 0:W], AF.Tanh,
                                             scale=scl_main)
                    else:
                        b = segs[0][0]
                        nc.scalar.activation(th[:, ds(ht % 2, 1), :],
                                             pa1[:], AF.Tanh,
                                             bias=bias_sb[:, ds(ht * BL + b, 1)],
                                             scale=scl_main)
                    if ht % 2 == 1:
                        pending.append((th, ht // 2))
                        if len(pending) > delay:
                            pth, j = pending.pop(0)
                            nc.tensor.matmul(psc[:], w2pad[:, ds(2 * j, 2), :],
                                             pth[:, 0:2, :],
                                             start=(j == 0),
                                             stop=(j == HT // 2 - 1),
                                             perf_mode=DR)
                for pth, j in pending:
                    nc.tensor.matmul(psc[:], w2pad[:, ds(2 * j, 2), :],
                                     pth[:, 0:2, :], start=(j == 0),
                                     stop=(j == HT // 2 - 1), perf_mode=DR)
                # scores*s2 += mask * -1e30*s2 (padding slots killed too)
                nc.vector.tensor_add(scores_sb[0:1, ds(toff, W)], psc[0:1, :],
                                     mneg_sb[0:1, ds(toff, W)])

                # ---- softmax, pipelined per tile; dequant (1/s2) rides
                # the exp scale port. |scores| <= ||W2||_1 <= 32, so
                # exp(s - 40) never overflows and softmax is
                # shift-invariant -- no max-reduce needed.
                for (b, _, sw, doff, slot), so in zip(segs, _seg_offs(segs)):
                    nc.scalar.activation(exps[0:1, ds(doff, sw)],
                                         scores_sb[0:1, ds(doff, sw)],
                                         AF.Exp, bias=c40[0:1, 0:1],
                                         scale=scl_exp,
                                         accum_out=ssum[0:1, ds(slot, 1)])
                for b in range(BL):
                    if last_tile_of_b[b] != t:
                        continue
                    nc.vector.reduce_sum(rcp[0:1, ds(b, 1)],
                                         ssum[0:1, ds(b * kmax, kmax)],
                                         axis=mybir.AxisListType.X)
                    nc.vector.reciprocal(rcp[0:1, ds(b, 1)], rcp[0:1, ds(b, 1)])
                    for doff, cw, _, _ in chunks[b]:
                        nc.vector.tensor_scalar_mul(attn[0:1, ds(doff, cw)],
                                                    exps[0:1, ds(doff, cw)],
                                                    rcp[0:1, ds(b, 1)])
            # single output DMA once the last b's attn is scaled
            nc.sync.dma_start(out_ext[:], attn[0:1, :])

    nc.compile()
    _cached[key] = (nc, tiles, chunks, TOTAL, kmax)
    return _cached[key]


def _seg_offs(segs):
    offs = []
    o = 0
    for _, _, w, _, _ in segs:
        offs.append(o)
        o += w
    return offs


def _to_pit(a, free):
    """[IT*P, free] -> contiguous [P, IT, free]."""
    return np.ascontiguousarray(a.reshape(IT, P, free).transpose(1, 0, 2))


def kernel(hidden, encoder_outputs, mask, W1, b1, W2, b2):
    global LAST_RESULT

    mask = np.asarray(mask, dtype=bool)
    counts = (~mask).sum(axis=1)
    padb = min(-(-int(counts.max()) // 64) * 64, S)

    enc = np.asarray(encoder_outputs, dtype=np.float32)
    enc_t = np.transpose(enc, (1, 2, 0))               # [B, Hin, S] fp32 view
    s_e = 240.0 / max(float(np.abs(enc).max()), 1e-30)
    W1 = np.asarray(W1, dtype=np.float32)
    w1e_f = W1[:, :HIN].T                              # [Hin, H]
    s_w = 240.0 / max(float(np.abs(w1e_f).max()), 1e-30)
    w1h_f = W1[:, HIN:].T                              # [H, H]
    s_wh = 240.0 / max(float(np.abs(w1h_f).max()), 1e-30)
    hid = np.asarray(hidden, dtype=np.float32)
    s_hid = 240.0 / max(float(np.abs(hid).max()), 1e-30)
    w2_f = np.asarray(W2, dtype=np.float32).reshape(H)
    s_2 = 240.0 / max(float(np.abs(w2_f).max()), 1e-30)

    nc, tiles, chunks, TOTAL, kmax = _build(
        padb,
        float(np.float32(1.0 / (s_e * s_w))),
        float(np.float32(1.0 / (s_hid * s_wh))),
        float(np.float32(1.0 / s_2)),
    )

    w1e = _to_pit(np.clip(w1e_f * s_w, -240, 240).astype(F8), H)
    w1h = _to_pit(np.clip(w1h_f * s_wh, -240, 240).astype(F8), H)
    b1 = np.ascontiguousarray(np.asarray(b1, dtype=np.float32).reshape(H))
    w2p = np.zeros((P, HT, P), dtype=F8)
    w2p[:, :, 0] = np.clip(w2_f * s_2, -240, 240).astype(F8).reshape(HT, P).T
    w2p = np.ascontiguousarray(w2p)

    in_maps = []
    dev_idx = {}
    for c in range(N_CORES):
        enc_dev = np.zeros((HIN, TOTAL), dtype=F8)
        mneg = np.full(TOTAL, np.float32(-1e30) * np.float32(s_2),
                       dtype=np.float32)
        for b in range(BL):
            gb = c * BL + b
            idx = np.nonzero(~mask[gb])[0]
            cb = len(idx)
            encg = np.clip(enc_t[gb][:, idx] * s_e, -240, 240).astype(F8)
            cols = np.empty(cb, dtype=np.int64)
            for doff, cw, start, _ in chunks[b]:
                real = min(max(cb - start, 0), cw)
                if real > 0:
                    enc_dev[:, doff:doff + real] = encg[:, start:start + real]
                    mneg[doff:doff + real] = 0.0
                    cols[start:start + real] = np.arange(doff, doff + real)
            dev_idx[gb] = (idx, cols)
        hid_pad = np.zeros((H, 16), dtype=np.float32)
        hid_pad[:, 0:BL] = hid[c * BL:(c + 1) * BL].T * s_hid
        in_maps.append({
            "enc": _to_pit(enc_dev, TOTAL),
            "hiddent": _to_pit(np.clip(hid_pad, -240, 240).astype(F8), 16),
            "maskneg": mneg,
            "w1e": w1e,
            "w1h": w1h,
            "b1": b1,
            "w2p": w2p,
        })

    res = run_bass_kernel_spmd(nc, in_maps, core_ids=list(range(N_CORES)))
    LAST_RESULT = res
    out = np.zeros((B, S), dtype=np.float32)
    for c in range(N_CORES):
        attn_dev = res.results[c]["out"]
        for b in range(BL):
            gb = c * BL + b
            idx, cols = dev_idx[gb]
            out[gb, idx] = attn_dev[cols]
    return np.ascontiguousarray(out[:, None, :])


# revision 9
# speedup vs baseline: 2.3712x; 1.0661x over previous
"""Trainium2 Bass kernel for the attention-scoring MLP (nn_Attn):

    enc = encoder_outputs.transpose(1,0,2)          # [B,S,Hin]
    a1  = tanh(enc @ W1_enc.T + hidden @ W1_hid.T + b1)
    s   = a1 @ W2[0] (+ b2 -- dropped: softmax shift-invariant)
    s   = where(mask, -inf, s)
    out = softmax(s, axis=-1)[:, None, :]           # [B,1,S]

Sharding: data-parallel over batch B=32 across 8 NeuronCores (4 rows
each), weights replicated, no collectives.

Levers over the naive dense form:
  1. Mask compaction. ~half the sequence positions are masked to -inf
     before the softmax, so their scores are never needed. The host
     gathers, per batch row, only the unmasked enc columns (padded to a
     uniform per-row capacity, multiple of 64) and the kernel computes
     scores only for those. Padded slots carry an additive -1e30 so
     their exp is exactly 0; the host scatters the compact attn back to
     full [B,1,S] with zeros at masked positions. Sub-512 per-row tails
     are packed together into shared tiles.
  2. fp8 (e4m3) DoubleRow matmuls everywhere on the PE: operands are
     absmax-scaled to fp8 on the host, pairs of 128-deep k-subtiles are
     fed per matmul (2 rows/cycle), and dequant scales ride the
     ScalarEngine activation scale ports. Accumulation stays fp32 in
     PSUM.
  3. One DMA per tensor/tile (3D access patterns, host pre-laid as
     [128, k-subtile, col]): DMA trigger instructions cost ~600ns on
     the issuing engine, so per-subtile DMAs serialized ~25us of
     startup.
"""

import numpy as np
import ml_dtypes

import concourse.bass as bass
import concourse.tile as tile
from concourse import bacc, mybir
from concourse.bass import ds, ts
from concourse.bass_utils import run_bass_kernel_spmd
from concourse.masks import make_identity

N_CORES = 8
B, S, HIN, H = 32, 1024, 1024, 1024
BL = B // N_CORES          # local batch rows per core
P = 128                    # partitions
IT = HIN // P              # contraction subtiles (128 deep each)
HT = H // P                # output-feature tiles
F32 = mybir.dt.float32
BF16 = mybir.dt.bfloat16
FP8 = mybir.dt.float8e4
AF = mybir.ActivationFunctionType
DR = mybir.MatmulPerfMode.DoubleRow
BF = ml_dtypes.bfloat16
F8 = ml_dtypes.float8_e4m3   # TRN fp8e4: IEEE e4m3, max finite +-240

_cached = {}
LAST_RESULT = None  # BassKernelResults of the most recent run (for test harness)


def _structure(padb):
    """Tile structure for per-row compact width `padb` (multiple of 64).

    Returns (tiles, chunks, total, kmax):
      tiles:  ordered list; each tile is (width, [(b, start_in_b, seg_w,
              dev_off, slot)]).  Order: b0's full-width chunks, then the
              shared tail tiles (packed sub-512 per-b tails), then
              b1..b3's full chunks -- so phase A overlaps a big tile and
              each b's softmax finalization lands right after its last
              chunk.
      chunks: per b, list of (dev_off, width, start_in_b, slot) in
              ascending start order.
      total:  device column count (sum of tile widths).
    """
    total = BL * padb
    ntiles = -(-total // 512)
    if padb <= 512:
        per_b = [padb]
    elif ntiles == BL + 1:
        # equalize widths so no tile is so narrow that DR LDWEIGHTS
        # (~134ns/matmul) stops hiding behind the matmul itself
        w0 = -(-total // (BL + 1) // 16) * 16
        per_b = [w0, padb - w0]
    else:
        per_b = [512] * (padb // 512)
        if padb % 512:
            per_b.append(padb % 512)
    kmax = len(per_b)

    tail_w = per_b[-1] * BL if (kmax > 1 and per_b[-1] * BL <= 512) else (
        per_b[-1] if kmax > 1 else None)
    tail_seg = per_b[-1] if kmax > 1 else None
    n_main = kmax - 1 if tail_seg is not None else kmax

    tiles = []
    chunks = [[] for _ in range(BL)]
    dev = 0

    starts = [sum(per_b[:k]) for k in range(kmax)]

    def add_main(b):
        nonlocal dev
        for k in range(n_main):
            w = per_b[k]
            tiles.append((w, [(b, starts[k], w, dev, b * kmax + k)]))
            chunks[b].append((dev, w, starts[k], b * kmax + k))
            dev += w

    add_main(0)
    if tail_seg is not None:
        group, gw = [], 0
        start_in_b = sum(per_b[:-1])
        for b in range(BL):
            if gw + tail_seg > 512:
                tiles.append((gw, group))
                group, gw = [], 0
            group.append((b, start_in_b, tail_seg, dev, b * kmax + kmax - 1))
            chunks[b].append((dev, tail_seg, start_in_b, b * kmax + kmax - 1))
            dev += tail_seg
            gw += tail_seg
        tiles.append((gw, group))
    for b in range(1, BL):
        add_main(b)
    for b in range(BL):
        chunks[b].sort(key=lambda c: c[2])
    return tiles, chunks, dev, kmax


def _build(padb, scl_main, scl_hid, scl_exp):
    key = (padb, scl_main, scl_hid, scl_exp)
    if key in _cached:
        return _cached[key]

    tiles, chunks, TOTAL, kmax = _structure(padb)
    last_tile_of_b = {}
    for t, (_, segs) in enumerate(tiles):
        for b, *_ in segs:
            last_tile_of_b[b] = t
    NTILES = len(tiles)
    # shared (multi-segment) tiles get their own bias_tail column block
    shared_idx = {t: i for i, (t, (w, segs)) in
                  enumerate((t, x) for t, x in enumerate(tiles)
                            if len(x[1]) > 1)}
    NSH = len(shared_idx)
    tailw = [tiles[t][0] for t in shared_idx]
    TW = max(tailw) if tailw else 0

    nc = bacc.Bacc("TRN2", target_bir_lowering=False, debug=False,
                   num_devices=N_CORES)

    # compact encT, device-column layout, pre-arranged [p, it, col]
    enc_ext = nc.dram_tensor("enc", [P, IT, TOTAL], FP8, kind="ExternalInput").ap()
    # hiddenT, [p, it, b] padded to 16 along b
    hidt_ext = nc.dram_tensor("hiddent", [P, IT, 16], FP8, kind="ExternalInput").ap()
    mneg_ext = nc.dram_tensor("maskneg", [TOTAL], F32, kind="ExternalInput").ap()
    # W1 split, transposed, fp8-scaled, pre-arranged [p, it, h]
    w1e_ext = nc.dram_tensor("w1e", [P, IT, H], FP8, kind="ExternalInput").ap()
    w1h_ext = nc.dram_tensor("w1h", [P, IT, H], FP8, kind="ExternalInput").ap()
    b1_ext = nc.dram_tensor("b1", [H], F32, kind="ExternalInput").ap()
    # W2 padded stationary [p, ht, 128] (col 0 = scaled w2 chunk, rest 0)
    w2p_ext = nc.dram_tensor("w2p", [P, HT, P], FP8, kind="ExternalInput").ap()
    out_ext = nc.dram_tensor("out", [TOTAL], F32, kind="ExternalOutput").ap()

    with tile.TileContext(nc) as tc:
        with (
            tc.tile_pool(name="consts", bufs=1) as consts,
            tc.tile_pool(name="encp", bufs=3) as encp,
            tc.tile_pool(name="thp", bufs=4) as thp,
            tc.tile_pool(name="pap", bufs=2, space="PSUM") as pap,
            tc.tile_pool(name="pscp", bufs=2, space="PSUM") as pscp,
            tc.tile_pool(name="psA", bufs=1, space="PSUM") as psA,
            tc.tile_pool(name="psT", bufs=2, space="PSUM") as psTp,
        ):
            # ---- PE warmup: junk matmuls with no DMA deps so the HAM
            # clock-gate is already at 8/8 when the real matmuls arrive.
            warm_sb = consts.tile([P, 512], BF16)
            nc.gpsimd.memset(warm_sb[:], 0.0)
            warm_ps = pap.tile([P, 512], F32, tag="pa1")
            for _ in range(10):
                nc.tensor.matmul(warm_ps[:], warm_sb[:, 0:P], warm_sb[:],
                                 start=True, stop=True)

            # ---- resident weights/constants (one DMA each) ----
            # Emission order = ring service order: first-needed first.
            hT_sb = consts.tile([P, IT, 16], FP8)
            nc.sync.dma_start(hT_sb[:], hidt_ext[:])
            w1e_sb = consts.tile([P, IT, H], FP8)
            nc.sync.dma_start(w1e_sb[:], w1e_ext[:])
            # phase A's w1h leads the scalar ring so the bias chain can
            # start as soon as the PE warmup finishes; the first tile's
            # enc follows (overlapped with w1e on sync).
            w1h_sb = consts.tile([P, IT, H], FP8)
            nc.scalar.dma_start(w1h_sb[:], w1h_ext[:])
            w0 = tiles[0][0]
            enc0_sb = consts.tile([P, IT, w0], FP8, tag="enc0")
            nc.scalar.dma_start(enc0_sb[:],
                                enc_ext[:, :, ds(tiles[0][1][0][3], w0)])
            b1T_sb = consts.tile([P, HT], F32)
            nc.sync.dma_start(b1T_sb[:], b1_ext.rearrange("(ht p) -> p ht", p=P))
            w2pad = consts.tile([P, HT, P], FP8)
            nc.sync.dma_start(w2pad[:], w2p_ext[:])
            mneg_sb = consts.tile([1, TOTAL], F32)
            nc.sync.dma_start(mneg_sb[:], mneg_ext[:])
            ident_sb = consts.tile([BL, BL], F32)
            make_identity(nc, ident_sb[:])

            bias_sb = consts.tile([P, HT * BL], F32)   # [p, ht*BL+b]
            bias_tail = (consts.tile([P, NSH * HT * TW], F32, name="bias_tail")
                         if TW else None)
            hterm_sb = consts.tile([BL, H], F32)
            scores_sb = consts.tile([1, TOTAL], F32)
            c40 = consts.tile([1, 1], F32)
            nc.gpsimd.memset(c40[:], -40.0)
            exps = consts.tile([1, TOTAL], F32)
            ssum = consts.tile([1, BL * kmax], F32)
            rcp = consts.tile([1, BL], F32)
            attn = consts.tile([1, TOTAL], F32)
            ttmp = consts.tile([P, TW], F32, name="ttmp") if TW else None

            # ---- phase A: h_term[b,h] = hidden @ W1_hid.T (fp8);
            # bias = h_termT + b1T, dequant via the Copy scale port.
            pht = psA.tile([16, H], F32)
            for kp in range(IT // 2):
                lhs = hT_sb[:, ds(2 * kp, 2), :]
                nc.tensor.matmul(pht[:, 0:512], lhs,
                                 w1h_sb[:, ds(2 * kp, 2), 0:512],
                                 start=(kp == 0), stop=(kp == IT // 2 - 1),
                                 perf_mode=DR)
                nc.tensor.matmul(pht[:, 512:H], lhs,
                                 w1h_sb[:, ds(2 * kp, 2), 512:H],
                                 start=(kp == 0), stop=(kp == IT // 2 - 1),
                                 perf_mode=DR)
            nc.scalar.activation(hterm_sb[:], pht[0:BL, :], AF.Copy)
            # bias = h_termT*dequant + b1T in one two-op tensor_scalar
            for ht in range(HT):
                ptT = psTp.tile([P, BL], F32)
                nc.tensor.transpose(ptT[:], hterm_sb[:, ts(ht, P)], ident_sb[:])
                nc.vector.tensor_scalar(bias_sb[:, ts(ht, BL)], ptT[:],
                                        scl_hid, b1T_sb[:, ds(ht, 1)],
                                        mybir.AluOpType.mult,
                                        mybir.AluOpType.add)
            # per-column bias for the shared-tail tiles (vector-added
            # there, since segments of different b share one tile)
            # pre-scaled by 1/scl_main: the shared-tile tanh computes
            # tanh((pa1 + bias_tail) * scl_main)
            for t, sidx in shared_idx.items():
                segs0 = tiles[t][1]
                offs0 = _seg_offs(segs0)
                for ht in range(HT):
                    for (b, _, sw, _, _), so in zip(segs0, offs0):
                        nc.vector.tensor_scalar(
                            bias_tail[:, ds((sidx * HT + ht) * TW + so, sw)],
                            warm_sb[:, ds(0, sw)],
                            bias_sb[:, ds(ht * BL + b, 1)],
                            1.0 / scl_main,
                            mybir.AluOpType.add,
                            mybir.AluOpType.mult)

            # ---- main loop over compact column tiles
            for t, (W, segs) in enumerate(tiles):
                toff = segs[0][3]
                shared = len(segs) > 1
                if t == 0:
                    enc_sb = enc0_sb
                else:
                    enc_sb = encp.tile([P, IT, W], FP8, tag="enc")
                    eng = nc.scalar if t == 1 else nc.sync
                    eng.dma_start(enc_sb[:], enc_ext[:, :, ds(toff, W)])
                psc = pscp.tile([P, W], F32, tag="psc")
                # Delay the scores matmuls (issued per ht-pair) so a late
                # bias / tanh never stalls the in-order PE.
                delay = 2 if t == 0 else 1
                pending = []
                th = None
                for ht in range(HT):
                    pa1 = pap.tile([P, W], F32, tag="pa1")
                    for kp in range(IT // 2):
                        nc.tensor.matmul(
                            pa1[:],
                            w1e_sb[:, ds(2 * kp, 2), ds(ht * P, P)],
                            enc_sb[:, ds(2 * kp, 2), :],
                            start=(kp == 0), stop=(kp == IT // 2 - 1),
                            perf_mode=DR,
                        )
                    if ht % 2 == 0:
                        th = thp.tile([P, 2, W], FP8)
                    if shared:
                        nc.vector.tensor_add(
                            ttmp[:, 0:W], pa1[:],
                            bias_tail[:, ds((shared_idx[t] * HT + ht) * TW, W)])
                        nc.scalar.activation(th[:, ds(ht % 2, 1), :],
                                             ttmp[:, 0:W], AF.Tanh,
                                             scale=scl_main)
                    else:
                        b = segs[0][0]
                        nc.scalar.activation(th[:, ds(ht % 2, 1), :],
                                             pa1[:], AF.Tanh,
                                             bias=bias_sb[:, ds(ht * BL + b, 1)],
                                             scale=scl_main)
                    if ht % 2 == 1:
                        pending.append((th, ht // 2))
                        if len(pending) > delay:
                            pth, j = pending.pop(0)
                            nc.tensor.matmul(psc[:], w2pad[:, ds(2 * j, 2), :],
                                             pth[:, 0:2, :],
                                             start=(j == 0),
                                             stop=(j == HT // 2 - 1),
                                             perf_mode=DR)
                for pth, j in pending:
                    nc.tensor.matmul(psc[:], w2pad[:, ds(2 * j, 2), :],
                                     pth[:, 0:2, :], start=(j == 0),
                                     stop=(j == HT // 2 - 1), perf_mode=DR)
                # scores*s2 += mask * -1e30*s2 (padding slots killed too)
                nc.vector.tensor_add(scores_sb[0:1, ds(toff, W)], psc[0:1, :],
                                     mneg_sb[0:1, ds(toff, W)])

                # ---- softmax, pipelined per tile; dequant (1/s2) rides
                # the exp scale port. |scores| <= ||W2||_1 <= 32, so
                # exp(s - 40) never overflows and softmax is
                # shift-invariant -- no max-reduce needed.
                for (b, _, sw, doff, slot), so in zip(segs, _seg_offs(segs)):
                    nc.scalar.activation(exps[0:1, ds(doff, sw)],
                                         scores_sb[0:1, ds(doff, sw)],
                                         AF.Exp, bias=c40[0:1, 0:1],
                                         scale=scl_exp,
                                         accum_out=ssum[0:1, ds(slot, 1)])
                for b in range(BL):
                    if last_tile_of_b[b] != t:
                        continue
                    nc.vector.reduce_sum(rcp[0:1, ds(b, 1)],
                                         ssum[0:1, ds(b * kmax, kmax)],
                                         axis=mybir.AxisListType.X)
                    nc.vector.reciprocal(rcp[0:1, ds(b, 1)], rcp[0:1, ds(b, 1)])
                    for doff, cw, _, _ in chunks[b]:
                        nc.vector.tensor_scalar_mul(attn[0:1, ds(doff, cw)],
                                                    exps[0:1, ds(doff, cw)],
                                                    rcp[0:1, ds(b, 1)])
            # single output DMA once the last b's attn is scaled
            nc.sync.dma_start(out_ext[:], attn[0:1, :])

    nc.compile()
    _cached[key] = (nc, tiles, chunks, TOTAL, kmax)
    return _cached[key]


def _seg_offs(segs):
    offs = []
    o = 0
    for _, _, w, _, _ in segs:
        offs.append(o)
        o += w
    return offs


def _to_pit(a, free):
    """[IT*P, free] -> contiguous [P, IT, free]."""
    return np.ascontiguousarray(a.reshape(IT, P, free).transpose(1, 0, 2))


def kernel(hidden, encoder_outputs, mask, W1, b1, W2, b2):
    global LAST_RESULT

    mask = np.asarray(mask, dtype=bool)
    counts = (~mask).sum(axis=1)
    padb = min(-(-int(counts.max()) // 64) * 64, S)

    enc = np.asarray(encoder_outputs, dtype=np.float32)
    enc_t = np.transpose(enc, (1, 2, 0))               # [B, Hin, S] fp32 view
    s_e = 240.0 / max(float(np.abs(enc).max()), 1e-30)
    W1 = np.asarray(W1, dtype=np.float32)
    w1e_f = W1[:, :HIN].T                              # [Hin, H]
    s_w = 240.0 / max(float(np.abs(w1e_f).max()), 1e-30)
    w1h_f = W1[:, HIN:].T                              # [H, H]
    s_wh = 240.0 / max(float(np.abs(w1h_f).max()), 1e-30)
    hid = np.asarray(hidden, dtype=np.float32)
    s_hid = 240.0 / max(float(np.abs(hid).max()), 1e-30)
    w2_f = np.asarray(W2, dtype=np.float32).reshape(H)
    s_2 = 240.0 / max(float(np.abs(w2_f).max()), 1e-30)

    nc, tiles, chunks, TOTAL, kmax = _build(
        padb,
        float(np.float32(1.0 / (s_e * s_w))),
        float(np.float32(1.0 / (s_hid * s_wh))),
        float(np.float32(1.0 / s_2)),
    )

    w1e = _to_pit(np.clip(w1e_f * s_w, -240, 240).astype(F8), H)
    w1h = _to_pit(np.clip(w1h_f * s_wh, -240, 240).astype(F8), H)
    b1 = np.ascontiguousarray(np.asarray(b1, dtype=np.float32).reshape(H))
    w2p = np.zeros((P, HT, P), dtype=F8)
    w2p[:, :, 0] = np.clip(w2_f * s_2, -240, 240).astype(F8).reshape(HT, P).T
    w2p = np.ascontiguousarray(w2p)

    in_maps = []
    dev_idx = {}
    for c in range(N_CORES):
        enc_dev = np.zeros((HIN, TOTAL), dtype=F8)
        mneg = np.full(TOTAL, np.float32(-1e30) * np.float32(s_2),
                       dtype=np.float32)
        for b in range(BL):
            gb = c * BL + b
            idx = np.nonzero(~mask[gb])[0]
            cb = len(idx)
            encg = np.clip(enc_t[gb][:, idx] * s_e, -240, 240).astype(F8)
            cols = np.empty(cb, dtype=np.int64)
            for doff, cw, start, _ in chunks[b]:
                real = min(max(cb - start, 0), cw)
                if real > 0:
                    enc_dev[:, doff:doff + real] = encg[:, start:start + real]
                    mneg[doff:doff + real] = 0.0
                    cols[start:start + real] = np.arange(doff, doff + real)
            dev_idx[gb] = (idx, cols)
        hid_pad = np.zeros((H, 16), dtype=np.float32)
        hid_pad[:, 0:BL] = hid[c * BL:(c + 1) * BL].T * s_hid
        in_maps.append({
            "enc": _to_pit(enc_dev, TOTAL),
            "hiddent": _to_pit(np.clip(hid_pad, -240, 240).astype(F8), 16),
            "maskneg": mneg,
            "w1e": w1e,
            "w1h": w1h,
            "b1": b1,
            "w2p": w2p,
        })

    res = run_bass_kernel_spmd(nc, in_maps, core_ids=list(range(N_CORES)))
    LAST_RESULT = res
    out = np.zeros((B, S), dtype=np.float32)
    for c in range(N_CORES):
        attn_dev = res.results[c]["out"]
        for b in range(BL):
            gb = c * BL + b
            idx, cols = dev_idx[gb]
            out[gb, idx] = attn_dev[cols]
    return np.ascontiguousarray(out[:, None, :])
